# revision 1
# baseline (speedup 1.0000x reference)
"""Trainium2 Bass kernel for nn_Attention (ViT-style attention block).

Reference computation (per batch b, head h):
    qkv  = x @ qkv_weight.T + [q_bias, 0, v_bias]        # [B,N,3C]
    q,k,v split into heads of HD=64;  q *= HD**-0.5
    S    = q @ k.T + relative_position_bias[h]           # [N,N]
    P    = softmax(S, axis=-1)
    O    = P @ v                                         # [N,HD]
    out  = concat_heads(O) @ proj_weight.T + proj_bias   # [B,N,C]

Sharding: pure data-parallel over the batch dim: 16 batches -> 2 per core
across 8 NeuronCores.  Each core gets its own x-shard plus replicated
(host-pre-transposed) weights; outputs are concatenated on the host.

All matmul inputs are bf16 (PSUM accumulation in fp32); tolerance is 2e-2
and measured end-to-end error is ~6e-3.

Device dataflow (per core, B_l=2, T=B_l*N=1154 tokens):
  - qk-pass:  qkT[j, t] (feature-major) = Wqk^T.T @ x^T ; q rows pre-scaled
    by HD**-0.5 on the host (folded into Wq and q_bias).
  - v-pass:   v[t, j] (token-major) = x^T.T @ Wv^T, stored per head with a
    ones column appended: v_aug[t, (h, 0:65)] = [v_h | 1].
  - attention per (head-pair hp, batch): the two heads' S^T matmuls write the
    two halves of one 4-bank PSUM tile (PE row groups 0:63 / 64:127 run
    concurrently); ONE exp on ScalarE covers both heads, then P^T *=
    exp(bias^T) in one tensor_tensor per mt (host precomputes exp of the
    transposed bias; work split between VectorE and GpSimd).
    O'^T for both heads accumulates into the two halves of another PSUM tile
    (row 64 = softmax denominators via the ones column of v_aug).
  - normalization per head-pair: denominator rows collected into an SBUF
    tile via SWDGE, reciprocals on DVE (reciprocal_approx_fast - no ScalarE
    act-table thrash), bf16 partition-broadcast DMA, in-place multiply.
  - proj: out[t, jo] = O^T.T @ Wp^T + proj_bias.

bf16 DMA rule learned on HW: a DMA write run into SBUF must start 4-byte
aligned and may overhang its end by 2 bytes (min 4-byte write granularity)
-- every bf16 destination row here is padded so overhangs land in padding,
never in live data (a 2-byte pad column DMA clobbered its neighbor before).
"""

import numpy as np

B, N, C = 16, 577, 768
H, HD = 12, 64
SCALE = HD ** -0.5
NCORES = 8
BL = B // NCORES           # batches per core (2)
T = BL * N                 # tokens per core (1154)
NT_M = (N + 127) // 128    # m-tiles per batch (5: 4x128 + 65)
CT = C // 128              # 128-contraction tiles over C (6)
NP = N + 1                 # n padded to even (578) for matmul free dims
TP = BL * NP               # padded token rows (1156)

_CACHE = {}


def _chunks(total, limit=512):
    """Bank-aligned matmul free-dim chunks (each <= 512 fp32 = one PSUM bank)."""
    out = []
    pos = 0
    while pos < total:
        n = min(limit, total - pos)
        out.append((pos, n))
        pos += n
    return out


def _build():
    """Trace the Bass/Tile program once. Returns the Bass object."""
    import concourse.bass as bass
    import concourse.tile as tile
    from concourse import bacc, mybir
    from contextlib import ExitStack

    f32 = mybir.dt.float32
    bf16 = mybir.dt.bfloat16
    ALU = mybir.AluOpType
    ACTF = mybir.ActivationFunctionType

    nc = bacc.Bacc("TRN2", target_bir_lowering=False, debug=False)

    # ---- DRAM I/O ----
    xT_d = nc.dram_tensor("xT", [C, T], bf16, kind="ExternalInput").ap()
    wqk_d = nc.dram_tensor("wqkT", [C, 2 * C], bf16, kind="ExternalInput").ap()
    wv_d = nc.dram_tensor("wvT", [C, C], bf16, kind="ExternalInput").ap()
    wp_d = nc.dram_tensor("wpT", [C, C], bf16, kind="ExternalInput").ap()
    qb_d = nc.dram_tensor("qbT", [128, CT], f32, kind="ExternalInput").ap()
    vb_d = nc.dram_tensor("vbB", [128, C], f32, kind="ExternalInput").ap()
    pb_d = nc.dram_tensor("pbB", [128, C], f32, kind="ExternalInput").ap()
    bt_d = nc.dram_tensor("BT", [H, N, N], bf16, kind="ExternalInput").ap()
    ones_d = nc.dram_tensor(
        "ones", [128, BL * NT_M, H, 2], bf16, kind="ExternalInput"
    ).ap()
    out_d = nc.dram_tensor("out", [TP, C], bf16, kind="ExternalOutput").ap()

    def r(x):
        return x

    with tile.TileContext(nc) as tc, ExitStack() as ctx:
        const = ctx.enter_context(tc.tile_pool(name="const", bufs=1))
        persist = ctx.enter_context(tc.tile_pool(name="persist", bufs=1))

        qb_sb = const.tile([128, CT], f32)
        vb_sb = const.tile([128, C], f32)
        pb_sb = const.tile([128, C], f32)

        # Persistent activations
        qk_sb = persist.tile([128, 2 * CT, BL * NP], bf16)      # q^T | k^T
        v_sb = persist.tile([128, BL * NT_M, H, HD + 2], bf16)  # v_aug
        ot_sb = persist.tile([128, CT, BL, NP], bf16)           # O^T (padded)
        dall = persist.tile([4, H // 2 + 1, NP], f32)           # denominators

        # Phase-2/3 SBUF pools are created (and their first tiles allocated)
        # BEFORE phase 1 closes: tiles allocated after phase 1 would alias
        # its xT/wqk/wv SBUF, and their DMAs would inherit a wait on
        # phase-1's last PE reader (observed: rings idle 40us->95us).
        wpp = ctx.enter_context(tc.tile_pool(name="wpp", bufs=1))
        btp = ctx.enter_context(tc.tile_pool(name="btp", bufs=2))
        pp = ctx.enter_context(tc.tile_pool(name="pp", bufs=7))
        sums = ctx.enter_context(tc.tile_pool(name="sums", bufs=3))
        nrm = ctx.enter_context(tc.tile_pool(name="nrm", bufs=2))
        bcst = ctx.enter_context(tc.tile_pool(name="bcst", bufs=4))
        rdp = ctx.enter_context(tc.tile_pool(name="rdp", bufs=2, space="DRAM"))
        oscr = ctx.enter_context(tc.tile_pool(name="oscr", bufs=2))
        outp = ctx.enter_context(tc.tile_pool(name="outp", bufs=3))

        def load_btpair(hp):
            btpair = btp.tile([128, 2, NT_M, NP], bf16)
            for par in range(2):
                h = 2 * hp + par
                ring = nc.sync if par == 0 else nc.scalar
                ring.dma_start(
                    out=btpair[:, par, 0:4, 0:N],
                    in_=bt_d[h, 0:512, :].rearrange("(a p) n -> p a n", p=128),
                )
                ring.dma_start(
                    out=btpair[0:65, par, 4, 0:N], in_=bt_d[h, 512:N, :]
                )
            return btpair

        # ---------------- Phase 1: qkv projections ----------------
        with ExitStack() as p1:
            xp = p1.enter_context(tc.tile_pool(name="xp", bufs=1))
            wvp = p1.enter_context(tc.tile_pool(name="wvp", bufs=1))
            xT_sb = xp.tile([128, CT, T], bf16)
            wv_sb = wvp.tile([128, CT, C], bf16)

            # --- qk-pass (feature-major output) ---
            with ExitStack() as p1b:
                wqkp = p1b.enter_context(tc.tile_pool(name="wqkp", bufs=1))
                wqk_sb = wqkp.tile([128, CT, 2 * C], bf16)
                # per-ct transfers: multiple outstanding medium transfers
                # outperform one large one (measured). xT on SP; wqk split
                # across BOTH rings (it rate-limits the jt chains); wv after.
                for ct in range(CT):
                    ring = nc.scalar if ct < 3 else nc.sync
                    ring.dma_start(
                        out=wqk_sb[:, ct, :],
                        in_=wqk_d[ct * 128:(ct + 1) * 128, :],
                    )
                    if ct < 3:
                        nc.sync.dma_start(
                            out=xT_sb[:, ct, :],
                            in_=xT_d[ct * 128:(ct + 1) * 128, :],
                        )
                    else:
                        nc.scalar.dma_start(
                            out=xT_sb[:, ct, :],
                            in_=xT_d[ct * 128:(ct + 1) * 128, :],
                        )
                for ct in range(CT):
                    ring = nc.scalar if ct % 2 == 0 else nc.sync
                    ring.dma_start(
                        out=wv_sb[:, ct, :],
                        in_=wv_d[ct * 128:(ct + 1) * 128, :],
                    )
                nc.sync.dma_start(out=qb_sb[:], in_=qb_d)
                nc.sync.dma_start(out=vb_sb[:], in_=vb_d)
                nc.sync.dma_start(out=pb_sb[:], in_=pb_d)
                nc.gpsimd.dma_start(out=v_sb[:, :, :, HD:HD + 2], in_=ones_d[:])
                # Zero padding columns on a compute engine (NOT a 2-byte DMA,
                # which would clobber the adjacent real column - HW race).
                nc.gpsimd.memset(
                    qk_sb[:].rearrange("p j (b n) -> p j b n", n=NP)[
                        :, :, :, N:N + 1
                    ],
                    0.0,
                )
                nc.gpsimd.memset(ot_sb[:, :, :, N:N + 1], 0.0)
                # Allocate + prefetch proj weights and the first two bias
                # pairs NOW, while phase-1 tiles are live: tiles allocated
                # after phase 1 closes alias its SBUF and their DMAs inherit
                # a wait on phase-1's last PE reader (observed 50us stall).
                wp_sb_l = wpp.tile([128, CT, C], bf16)
                nc.scalar.dma_start(
                    out=wp_sb_l[:], in_=wp_d.rearrange("(a p) j -> p a j", p=128)
                )
                bt_prefetch = [load_btpair(0), load_btpair(1)]
                qkps = p1b.enter_context(
                    tc.tile_pool(name="qkps", bufs=2, space="PSUM")
                )
                vb_v = vb_sb[:].rearrange("p (h d) -> p h d", d=HD)

                def qk_chain(jt):
                    ps = qkps.tile([128, T], f32, tag="p1")
                    for ct in range(CT):
                        for (t0, tn) in _chunks(T):
                            nc.tensor.matmul(
                                ps[:, t0:t0 + tn],
                                lhsT=r(wqk_sb[:, ct, jt * 128:(jt + 1) * 128]),
                                rhs=r(xT_sb[:, ct, t0:t0 + tn]),
                                start=(ct == 0),
                                stop=(ct == CT - 1),
                            )
                    qk_dst = qk_sb[:, jt, :].rearrange(
                        "p (b n) -> p b n", n=NP
                    )[:, :, 0:N]
                    ps_v = ps[:].rearrange("p (b n) -> p b n", n=N)
                    if jt < CT:
                        # q: add host-pre-scaled bias (per feature = partition)
                        nc.vector.tensor_scalar(
                            out=qk_dst,
                            in0=ps_v,
                            scalar1=qb_sb[:, jt:jt + 1],
                            scalar2=None,
                            op0=ALU.add,
                        )
                    else:
                        nc.vector.tensor_copy(qk_dst, ps_v)

                def v_chain(b):
                    for mt in range(NT_M):
                        mp = min(128, N - mt * 128)
                        t0 = b * N + mt * 128
                        idx = b * NT_M + mt
                        ps = qkps.tile([128, T], f32, tag="p1")
                        for ct in range(CT):
                            for (j0, jn) in _chunks(C):
                                nc.tensor.matmul(
                                    ps[0:mp, j0:j0 + jn],
                                    lhsT=r(xT_sb[:, ct, t0:t0 + mp]),
                                    rhs=r(wv_sb[:, ct, j0:j0 + jn]),
                                    start=(ct == 0),
                                    stop=(ct == CT - 1),
                                )
                        nc.vector.tensor_add(
                            v_sb[0:mp, idx, :, 0:HD],
                            ps[0:mp, 0:C].rearrange("p (h d) -> p h d", d=HD),
                            vb_v[0:mp],
                        )

                # jt chains consume wqk halves in DMA arrival order; the
                # v-pass runs last (wv lands behind xT on the sync ring).
                for jt in range(2 * CT):
                    qk_chain(jt)
                v_chain(0)
                v_chain(1)


        # ---------------- Phase 2: attention ----------------
        with ExitStack() as p2:
            # one PSUM pool: S-pair tiles and O-pair tiles share it.
            # [128, 2, 1024] f32 = 4 banks; bufs=2 = all 8 banks.
            aps = p2.enter_context(tc.tile_pool(name="aps", bufs=2, space="PSUM"))

            def s_phase(hp, b, btpair):
                """Paired S^T matmuls for both heads of hp (PE row groups
                0:63/64:127 run concurrently) into the two halves of one PSUM
                tile; ONE exp for both heads per mt; one bias-mult per mt.
                Returns the 5 P-pair tiles."""
                qTs, kTs = [], []
                for par in range(2):
                    base = par * 64
                    qTs.append(qk_sb[base:base + 64, hp, b * NP:(b + 1) * NP])
                    kTs.append(qk_sb[base:base + 64, CT + hp, b * NP:(b + 1) * NP])
                pts = []
                for mt in range(NT_M):
                    mp = min(128, N - mt * 128)
                    sp = aps.tile([128, 2, 1024], f32, tag="aps")
                    for par in range(2):
                        for (n0, nn) in _chunks(NP):
                            nc.tensor.matmul(
                                sp[0:mp, par, n0:n0 + nn],
                                lhsT=r(kTs[par][:, mt * 128:mt * 128 + mp]),
                                rhs=r(qTs[par][:, n0:n0 + nn]),
                                start=True,
                                stop=True,
                            )
                    pt = pp.tile([128, 2, NP], bf16, tag="ptile")
                    nc.scalar.activation(
                        pt[0:mp, :, :], sp[0:mp, :, 0:NP], ACTF.Exp
                    )
                    # all bias mults on DVE: Pool is ~6x slower per element
                    # and its trigger work at hp boundaries delayed O(mt=1)
                    nc.vector.tensor_mul(
                        pt[0:mp, :, 0:N], pt[0:mp, :, 0:N],
                        btpair[0:mp, :, mt, 0:N],
                    )
                    pts.append(pt)
                return pts

            def o_phase(hp, b, pts, drow=None, dcol=None):
                """O'^T for both heads into the two halves of one PSUM tile;
                row 64 = softmax denominators (ones column of v_aug)."""
                ou = aps.tile([128, 2, 1024], f32, tag="aps")
                for mt in range(NT_M):
                    mp = min(128, N - mt * 128)
                    for par in range(2):
                        h = 2 * hp + par
                        for (n0, nn) in _chunks(NP):
                            nc.tensor.matmul(
                                ou[0:HD + 2, par, n0:n0 + nn],
                                lhsT=r(v_sb[0:mp, b * NT_M + mt, h, :]),
                                rhs=r(pts[mt][0:mp, par, n0:n0 + nn]),
                                start=(mt == 0),
                                stop=(mt == NT_M - 1),
                            )
                # ONE DVE copy drains both heads' O' AND the denominator
                # row (rows are partition-parallel, so 0:65 costs the same
                # as 0:64) -- keeps the denominator copy OFF ScalarE, which
                # is ~83% busy with the exp chain. The SWDGE dall DMA casts
                # the bf16 denominators back to f32 (HWDGE can't cast).
                if drow is None:
                    drow, dcol = 2 * b, hp
                sc = oscr.tile([HD + 1, 2, NP], bf16)
                nc.vector.tensor_copy(sc[:, :, 0:N], ou[0:HD + 1, :, 0:N])
                nc.gpsimd.dma_start(
                    out=dall[drow:drow + 2, dcol, 0:N],
                    in_=sc[HD:HD + 1, :, 0:N],
                )
                nc.sync.dma_start(
                    out=ot_sb[0:64, hp, b, 0:N], in_=sc[0:64, 0, 0:N]
                )
                nc.sync.dma_start(
                    out=ot_sb[64:128, hp, b, 0:N], in_=sc[0:64, 1, 0:N]
                )

            def normalize(hp, dcol, ni, bs, fast=False):
                """Reciprocals for batches `bs` (rows 2b, 2b+1 = par0/par1 at
                dall[:, dcol]) on DVE; the SWDGE bounce DMA casts f32->bf16
                itself; ONE [128, N] broadcast + ONE full-height multiply per
                batch. fast=True (last head-pair) runs the multiply on DVE
                (it gates proj); others run on Pool to keep DVE free."""
                rall = nrm.tile([4, NP], f32, tag="rall")
                nc.vector.reciprocal_approx_fast(
                    rall[0:ni, 0:N], dall[0:ni, dcol, 0:N]
                )
                # partition-broadcast DMA needs a DRAM source: tiny bounce
                # (SWDGE casts f32 -> bf16 during the copy)
                rdr = rdp.tile([4, NP], bf16, tag="rdr")
                nc.gpsimd.dma_start(out=rdr[0:ni, 0:N], in_=rall[0:ni, 0:N])
                for j, b in enumerate(bs):
                    bc = bcst.tile([128, NP], bf16, tag="bc")
                    for par in range(2):
                        nc.gpsimd.dma_start(
                            out=bc[par * 64:par * 64 + 64, 0:N],
                            in_=rdr[2 * j + par:2 * j + par + 1, 0:N]
                            .broadcast_to([64, N]),
                        )
                    eng = nc.vector if fast else nc.gpsimd
                    eng.tensor_mul(
                        ot_sb[:, hp, b, 0:N],
                        ot_sb[:, hp, b, 0:N],
                        bc[:, 0:N],
                    )

            bts = {0: bt_prefetch[0], 1: bt_prefetch[1]}
            for hp in range(H // 2):
                # issue the NEXT pair's load now: its pool buffer (freed at
                # hp-1's end) is available, so the transfer overlaps all of
                # this head-pair's compute instead of starting at hp+1.
                if hp + 1 < H // 2 and hp + 1 not in bts:
                    bts[hp + 1] = load_btpair(hp + 1)
                btpair = bts.pop(hp)
                last = hp == H // 2 - 1
                for b in range(BL):
                    pts = s_phase(hp, b, btpair)
                    if last:
                        # final head-pair: each batch gets its own dall column
                        # (engine base-partition must be 0) and normalizes
                        # immediately so proj's dependency resolves early
                        o_phase(hp, b, pts, drow=0, dcol=hp + b)
                        normalize(hp, hp + b, 2, [b], fast=True)
                    else:
                        o_phase(hp, b, pts)
                if not last:
                    normalize(hp, hp, 4, [0, 1])

        # ---------------- Phase 3: output projection ----------------
        with ExitStack() as p3:
            pps = p3.enter_context(tc.tile_pool(name="pps", bufs=2, space="PSUM"))
            ntt = (TP + 127) // 128
            ot_flat = ot_sb[:].rearrange("p c b n -> p c (b n)")
            for tt in range(ntt):
                tp = min(128, TP - tt * 128)
                ps = pps.tile([128, C], f32)
                for ct in range(CT):
                    for (j0, jn) in _chunks(C):
                        nc.tensor.matmul(
                            ps[0:tp, j0:j0 + jn],
                            lhsT=r(ot_flat[:, ct, tt * 128:tt * 128 + tp]),
                            rhs=r(wp_sb_l[:, ct, j0:j0 + jn]),
                            start=(ct == 0),
                            stop=(ct == CT - 1),
                        )
                os = outp.tile([128, C], bf16)
                nc.vector.tensor_add(os[0:tp, :], ps[0:tp, :], pb_sb[0:tp, :])
                ring = nc.sync if tt % 2 == 0 else nc.scalar
                ring.dma_start(
                    out=out_d[tt * 128:tt * 128 + tp, :], in_=os[0:tp, :]
                )

    nc.compile()
    return nc


def _get_nc():
    if "nc" not in _CACHE:
        _CACHE["nc"] = _build()
    return _CACHE["nc"]


def _prep_inputs(x, relative_position_bias, qkv_weight, q_bias, v_bias,
                 proj_weight, proj_bias):
    """Host-side layout prep + per-core sharding. Returns list of in_maps."""
    import ml_dtypes

    f = np.float32
    bf = ml_dtypes.bfloat16
    x = np.asarray(x, f)
    bias = np.asarray(relative_position_bias, f)
    w = np.asarray(qkv_weight, f)
    qb = np.asarray(q_bias, f)
    vb = np.asarray(v_bias, f)
    wp = np.asarray(proj_weight, f)
    pb = np.asarray(proj_bias, f)

    wq_s = w[0:C] * f(SCALE)            # fold q scaling into weights/bias
    qb_s = qb * f(SCALE)
    wqkT = np.ascontiguousarray(np.concatenate([wq_s, w[C:2 * C]], 0).T.astype(bf))
    wvT = np.ascontiguousarray(w[2 * C:].T.astype(bf))
    wpT = np.ascontiguousarray(wp.T.astype(bf))
    qbT = np.ascontiguousarray(qb_s.reshape(CT, 128).T)
    vbB = np.ascontiguousarray(np.broadcast_to(vb, (128, C)))
    pbB = np.ascontiguousarray(np.broadcast_to(pb, (128, C)))
    BT = np.ascontiguousarray(
        np.exp(bias.transpose(0, 2, 1), dtype=np.float32).astype(bf)
    )

    ones = np.zeros((128, BL * NT_M, H, 2), dtype=bf)
    ones[:, :, :, 0] = 1.0
    shared = dict(wqkT=wqkT, wvT=wvT, wpT=wpT, qbT=qbT, vbB=vbB, pbB=pbB, BT=BT,
                  ones=ones)
    in_maps = []
    for c in range(NCORES):
        xs = x[c * BL:(c + 1) * BL].reshape(T, C)
        in_maps.append(dict(shared, xT=np.ascontiguousarray(xs.T.astype(bf))))
    return in_maps


def kernel(x, relative_position_bias, qkv_weight, q_bias, v_bias,
           proj_weight, proj_bias):
    from concourse import bass_utils

    in_maps = _prep_inputs(x, relative_position_bias, qkv_weight, q_bias,
                           v_bias, proj_weight, proj_bias)
    nc = _get_nc()
    res = bass_utils.run_bass_kernel_spmd(nc, in_maps, core_ids=list(range(NCORES)))
    out = np.concatenate(
        [res.results[c]["out"].reshape(BL, NP, C)[:, :N, :] for c in range(NCORES)],
        axis=0,
    )
    return out.astype(np.float32)



# revision 11
# speedup vs baseline: 1.0369x; 1.0369x over previous
"""Trainium2 Bass kernel for nn_Attention (ViT-style attention block).

Reference computation (per batch b, head h):
    qkv  = x @ qkv_weight.T + [q_bias, 0, v_bias]        # [B,N,3C]
    q,k,v split into heads of HD=64;  q *= HD**-0.5
    S    = q @ k.T + relative_position_bias[h]           # [N,N]
    P    = softmax(S, axis=-1)
    O    = P @ v                                         # [N,HD]
    out  = concat_heads(O) @ proj_weight.T + proj_bias   # [B,N,C]

Sharding: pure data-parallel over batch: 16 batches -> 2 per core across
8 NeuronCores; weights replicated; outputs concatenated on the host.

Single software-pipelined instruction stream (v2): qkv projection chains,
attention head-pair slots and the output projection are emitted in one
interleaved order so the PE never idles long enough to drop out of its
full-speed p-state (PE runs at 1.2GHz for ~3us after any idle gap; the
baseline's phase-separated version paid ~2x on every attention matmul).

Per attention slot (head-pair hp, batch b), queries split 512 + 65 so
every PSUM access stays inside one 2KB bank:
  - S^T pair matmuls (K=64) for both heads at tile_position (0,0)/(64,0).
  - ONE exp on ScalarE per mt covers both heads; P^T *= exp(bias^T)
    (host-precomputed) split between DVE and Pool.
  - O'^T accumulates over key tiles; v_aug = [1 | v] puts the softmax
    denominator in PSUM partition 0, so DVE reciprocal reads it directly
    (no SWDGE collect); f32 HWDGE bounce + partition-broadcast; normalization is an in-place multiply on Pool.
  - qkv/proj chains for FUTURE slots are emitted inside each slot as PE
    filler while ScalarE (exp, the per-slot rate limiter) catches up.

PSUM budget (8 banks): chain pool 2x[128,1024]f32 (4) + S-pair pool
2x[128,2,512]f32 (4); O accumulators borrow chain-pool tiles.

bf16 DMA rule learned on HW: a DMA write run into SBUF must start 4-byte
aligned and may overhang its end by 2 bytes -- all bf16 destination rows
here are padded so overhangs land in padding (580-wide bias rows, 578-wide
ot rows, 66-wide tail tiles).
"""

import numpy as np

B, N, C = 16, 577, 768
H, HD = 12, 64
SCALE = HD ** -0.5
NCORES = 8
BL = B // NCORES           # batches per core (2)
T = BL * N                 # tokens per core (1154)
NT_M = (N + 127) // 128    # key tiles per batch (5: 4x128 + 65)
CT = C // 128              # 128-contraction tiles over C (6)
HP = H // 2                # head pairs (6)
NP = N + 1                 # padded query stride (578)
TP = BL * NP               # padded token rows (1156)
NQ0 = 512                  # main-pass queries per batch
NQ1 = N - NQ0              # tail-pass queries (65)
NBP = 580                  # bias row stride (577 padded to 4B-aligned even)

_CACHE = {}


def _chunks(total, limit=512):
    out = []
    pos = 0
    while pos < total:
        n = min(limit, total - pos)
        out.append((pos, n))
        pos += n
    return out


def _build():
    """Trace the Bass/Tile program once. Returns the Bass object."""
    import concourse.bass as bass
    import concourse.tile as tile
    from concourse import bacc, mybir
    from contextlib import ExitStack

    f32 = mybir.dt.float32
    bf16 = mybir.dt.bfloat16
    ALU = mybir.AluOpType
    ACTF = mybir.ActivationFunctionType

    nc = bacc.Bacc("TRN2", target_bir_lowering=False, debug=False)

    # ---- DRAM I/O ----
    xT_d = nc.dram_tensor("xT", [C, T], bf16, kind="ExternalInput").ap()
    # qk weights partition-major + jt-block-major: one contiguous DMA per jt
    wqk_d = nc.dram_tensor(
        "wqkH", [128, 2 * CT, CT, 128], bf16, kind="ExternalInput"
    ).ap()
    wv_d = nc.dram_tensor("wvT", [C, C], bf16, kind="ExternalInput").ap()
    wp_d = nc.dram_tensor("wpT", [C, C], bf16, kind="ExternalInput").ap()
    qb_d = nc.dram_tensor("qbT", [128, CT], f32, kind="ExternalInput").ap()
    vb_d = nc.dram_tensor("vbB", [128, C], f32, kind="ExternalInput").ap()
    pb_d = nc.dram_tensor("pbB", [128, C], f32, kind="ExternalInput").ap()
    bt_d = nc.dram_tensor("BT", [H, N, NBP], bf16, kind="ExternalInput").ap()
    out_d = nc.dram_tensor("out", [TP, C], bf16, kind="ExternalOutput").ap()

    with tile.TileContext(nc) as tc, ExitStack() as ctx:
        const = ctx.enter_context(tc.tile_pool(name="const", bufs=1))
        persist = ctx.enter_context(tc.tile_pool(name="persist", bufs=1))

        qb_sb = const.tile([128, CT], f32)
        vb_sb = const.tile([128, C], f32)
        pb_sb = const.tile([128, C], f32)

        xT_sb = persist.tile([128, CT, T], bf16)
        wqk_sb = persist.tile([128, 2 * CT, CT, 128], bf16)
        wv_sb = persist.tile([128, CT, C], bf16)
        wp_sb = persist.tile([128, CT, C], bf16)
        qk_sb = persist.tile([128, 2 * CT, BL, NP], bf16)   # q^T | k^T
        v_sb = persist.tile([128, BL * NT_M, H, HD + 2], bf16)  # [1|v|pad]
        ot_sb = persist.tile([128, HP, BL, NP], bf16)       # O^T normalized

        # SBUF pools
        btp = ctx.enter_context(tc.tile_pool(name="btp", bufs=2))
        ptp = ctx.enter_context(tc.tile_pool(name="ptp", bufs=8))
        pttp = ctx.enter_context(tc.tile_pool(name="pttp", bufs=2))
        scp = ctx.enter_context(tc.tile_pool(name="scp", bufs=2))
        sctp = ctx.enter_context(tc.tile_pool(name="sctp", bufs=2))
        rap = ctx.enter_context(tc.tile_pool(name="rap", bufs=2))
        rdp = ctx.enter_context(tc.tile_pool(name="rdp", bufs=2, space="DRAM"))
        bcp = ctx.enter_context(tc.tile_pool(name="bcp", bufs=3))
        bctp = ctx.enter_context(tc.tile_pool(name="bctp", bufs=3))
        outp = ctx.enter_context(tc.tile_pool(name="outp", bufs=3))

        # PSUM pools: chains+O accumulators (2-bank tiles) and S pairs
        cps = ctx.enter_context(tc.tile_pool(name="cps", bufs=2, space="PSUM"))
        sps = ctx.enter_context(tc.tile_pool(name="sps", bufs=2, space="PSUM"))

        # ---------------- input DMAs (need-ordered) ----------------
        def ring(i):
            return nc.sync if i % 2 == 0 else nc.scalar

        # x^T for batch 0 first, then the first head-pair's qk weights
        for ct in range(CT):
            ring(ct).dma_start(
                out=xT_sb[:, ct, 0:N], in_=xT_d[ct * 128:(ct + 1) * 128, 0:N]
            )

        def load_wqk(jt):
            ring(jt).dma_start(out=wqk_sb[:, jt], in_=wqk_d[:, jt])

        load_wqk(0)
        load_wqk(CT)
        for ct in range(CT):
            ring(ct).dma_start(
                out=wv_sb[:, ct, :], in_=wv_d[ct * 128:(ct + 1) * 128, :]
            )
        for ct in range(CT):
            ring(ct).dma_start(
                out=xT_sb[:, ct, N:T], in_=xT_d[ct * 128:(ct + 1) * 128, N:T]
            )
        for jt in range(1, CT):
            load_wqk(jt)
            load_wqk(CT + jt)
        nc.sync.dma_start(out=qb_sb[:], in_=qb_d)
        nc.scalar.dma_start(out=vb_sb[:], in_=vb_d)
        nc.sync.dma_start(out=pb_sb[:], in_=pb_d)
        # ones column of v_aug (denominator trick): partition-0 output row
        nc.gpsimd.memset(v_sb[:, :, :, 0:1], 1.0)
        # ot pad column: keeps pad-token proj rows finite
        nc.gpsimd.memset(ot_sb[:, :, :, N:NP], 0.0)

        def load_bt(hp):
            """exp(bias)^T for head pair hp: [key-part, par, key-tile, q]."""
            bt = btp.tile([128, 2, NT_M, NBP], bf16)
            for par in range(2):
                h = 2 * hp + par
                ring(par).dma_start(
                    out=bt[:, par, 0:4, 0:N],
                    in_=bt_d[h, 0:512, 0:N].rearrange("(a p) n -> p a n", p=128),
                )
                ring(par + 1).dma_start(
                    out=bt[0:65, par, 4, 0:N], in_=bt_d[h, 512:N, 0:N]
                )
            return bt

        # ---------------- chains ----------------
        def qk_chain(jt, b):
            ps = cps.tile([128, 1024], f32, tag="c")
            for ct in range(CT):
                for (q0, qn) in _chunks(N):
                    nc.tensor.matmul(
                        ps[:, q0:q0 + qn],
                        lhsT=wqk_sb[:, jt, ct, :],
                        rhs=xT_sb[:, ct, b * N + q0:b * N + q0 + qn],
                        start=(ct == 0),
                        stop=(ct == CT - 1),
                    )
            if jt < CT:
                nc.vector.tensor_scalar(
                    out=qk_sb[:, jt, b, 0:N],
                    in0=ps[:, 0:N],
                    scalar1=qb_sb[:, jt:jt + 1],
                    scalar2=None,
                    op0=ALU.add,
                )
            else:
                nc.vector.tensor_copy(qk_sb[:, jt, b, 0:N], ps[:, 0:N])

        vb_v = vb_sb[:].rearrange("p (h d) -> p h d", d=HD)

        def v_chain(b, mt):
            mp = min(128, N - mt * 128)
            t0 = b * N + mt * 128
            ps = cps.tile([128, 1024], f32, tag="c")
            for ct in range(CT):
                for (j0, jn) in _chunks(C):
                    nc.tensor.matmul(
                        ps[0:mp, j0:j0 + jn],
                        lhsT=xT_sb[:, ct, t0:t0 + mp],
                        rhs=wv_sb[:, ct, j0:j0 + jn],
                        start=(ct == 0),
                        stop=(ct == CT - 1),
                    )
            nc.vector.tensor_add(
                v_sb[0:mp, b * NT_M + mt, :, 1:HD + 1],
                ps[0:mp, 0:C].rearrange("p (h d) -> p h d", d=HD),
                vb_v[0:mp],
            )

        def load_wp():
            nc.scalar.dma_start(
                out=wp_sb[:], in_=wp_d.rearrange("(a p) j -> p a j", p=128)
            )

        # ---------------- attention slot ----------------
        def slot(hp, b, bt, fillers):
            kT = qk_sb[:, CT + hp, b, :]
            qT = qk_sb[:, hp, b, :]
            # S main (queries 0:512): K=64 pairs on PE row groups 0/64
            pts = []
            for mt in range(NT_M):
                mp = min(128, N - mt * 128)
                sp = sps.tile([128, 2, NQ0], f32, tag="sp")
                for par in range(2):
                    p0 = 64 * par
                    nc.tensor.matmul(
                        sp[0:mp, par, :],
                        lhsT=kT[p0:p0 + 64, mt * 128:mt * 128 + mp],
                        rhs=qT[p0:p0 + 64, 0:NQ0],
                        start=True,
                        stop=True,
                        tile_position=(p0, 0),
                    )
                pt = ptp.tile([128, 2, NQ0], bf16)
                nc.scalar.activation(pt[0:mp], sp[0:mp], ACTF.Exp)
                # bias mult: split DVE/Pool to balance engine load
                eng = nc.vector if mt % 2 == 0 else nc.gpsimd
                eng.tensor_mul(pt[0:mp], pt[0:mp], bt[0:mp, :, mt, 0:NQ0])
                pts.append(pt)

            for f in fillers:
                f()

            # O main accumulation; partition 0 = softmax denominator
            ou = cps.tile([128, 1024], f32, name="ou", tag="c")
            ou = ou.rearrange("p (a n) -> p a n", n=NQ0)
            for mt in range(NT_M):
                mp = min(128, N - mt * 128)
                for par in range(2):
                    nc.tensor.matmul(
                        ou[0:HD + 1, par, :],
                        lhsT=v_sb[0:mp, b * NT_M + mt, 2 * hp + par, 0:HD + 1],
                        rhs=pts[mt][0:mp, par, :],
                        start=(mt == 0),
                        stop=(mt == NT_M - 1),
                    )

            # S tail (queries 512:577): all 5 key tiles in one PSUM tile
            spt = sps.tile([128, 2, NQ0], f32, tag="sp")
            for mt in range(NT_M):
                mp = min(128, N - mt * 128)
                for par in range(2):
                    p0 = 64 * par
                    nc.tensor.matmul(
                        spt[0:mp, par, mt * NQ1:(mt + 1) * NQ1],
                        lhsT=kT[p0:p0 + 64, mt * 128:mt * 128 + mp],
                        rhs=qT[p0:p0 + 64, NQ0:N],
                        start=True,
                        stop=True,
                        tile_position=(p0, 0),
                    )
            ptt = pttp.tile([128, 2, NT_M * NQ1], bf16)
            ptt_v = ptt[:].rearrange("p a (m q) -> p a m q", q=NQ1)
            nc.scalar.activation(
                ptt[:, :, 0:4 * NQ1], spt[:, :, 0:4 * NQ1], ACTF.Exp
            )
            nc.scalar.activation(
                ptt[0:65, :, 4 * NQ1:5 * NQ1],
                spt[0:65, :, 4 * NQ1:5 * NQ1],
                ACTF.Exp,
            )
            nc.vector.tensor_mul(
                ptt_v[:, :, 0:4, :], ptt_v[:, :, 0:4, :], bt[:, :, 0:4, NQ0:N]
            )
            nc.vector.tensor_mul(
                ptt_v[0:65, :, 4, :], ptt_v[0:65, :, 4, :],
                bt[0:65, :, 4, NQ0:N],
            )
            out_t = cps.tile([128, 1024], f32, name="out_t", tag="c")
            out_t = out_t.rearrange("p (a n) -> p a n", n=NQ0)
            for mt in range(NT_M):
                mp = min(128, N - mt * 128)
                for par in range(2):
                    nc.tensor.matmul(
                        out_t[0:HD + 1, par, 0:NQ1],
                        lhsT=v_sb[0:mp, b * NT_M + mt, 2 * hp + par, 0:HD + 1],
                        rhs=ptt[0:mp, par, mt * NQ1:(mt + 1) * NQ1],
                        start=(mt == 0),
                        stop=(mt == NT_M - 1),
                    )

            # reciprocals of the two denominator rows (bf16 out for the
            # HWDGE bounce -- no SWDGE cast needed)
            rall = rap.tile([1, 2, NBP], f32)
            nc.vector.reciprocal_approx_fast(rall[0:1, :, 0:NQ0], ou[0:1, :, :])
            nc.vector.reciprocal_approx_fast(
                rall[0:1, :, NQ0:N], out_t[0:1, :, 0:NQ1]
            )
            # PSUM -> SBUF drains (DVE casts f32->bf16)
            sc = scp.tile([128, 2, NQ0], bf16)
            nc.vector.tensor_copy(sc[0:HD + 1], ou[0:HD + 1])
            sct = sctp.tile([128, 2, HD + 2], bf16)
            nc.vector.tensor_copy(
                sct[0:HD + 1, :, 0:NQ1], out_t[0:HD + 1, :, 0:NQ1]
            )
            # bounce to DRAM (partition-broadcast needs a DRAM source)
            rd = rdp.tile([2, NBP], f32)
            nc.sync.dma_start(out=rd[:, 0:N], in_=rall[0:1, :, 0:N])
            bc = bcp.tile([128, NQ0], f32)
            bct = bctp.tile([128, HD + 2], f32)
            for par in range(2):
                p0 = 64 * par
                ring(par).dma_start(
                    out=bc[p0:p0 + 64, :],
                    in_=rd[par:par + 1, 0:NQ0].broadcast_to([64, NQ0]),
                )
                ring(par + 1).dma_start(
                    out=bct[p0:p0 + 64, 0:NQ1],
                    in_=rd[par:par + 1, NQ0:N].broadcast_to([64, NQ1]),
                )
                # head-half reorder: sc rows 1:65 -> ot partitions
                ring(par).dma_start(
                    out=ot_sb[p0:p0 + 64, hp, b, 0:NQ0], in_=sc[1:HD + 1, par, :]
                )
                ring(par + 1).dma_start(
                    out=ot_sb[p0:p0 + 64, hp, b, NQ0:N],
                    in_=sct[1:HD + 1, par, 0:NQ1],
                )
            # normalize in place on Pool (SBUF-only engine)
            nc.gpsimd.tensor_mul(
                ot_sb[:, hp, b, 0:NQ0], ot_sb[:, hp, b, 0:NQ0], bc[:]
            )
            nc.gpsimd.tensor_mul(
                ot_sb[:, hp, b, NQ0:N], ot_sb[:, hp, b, NQ0:N], bct[:, 0:NQ1]
            )

        # ---------------- emission schedule ----------------
        # prologue chains: everything slot (0,0) and (0,1) need
        qk_chain(0, 0)
        qk_chain(CT, 0)
        for mt in range(NT_M):
            v_chain(0, mt)
        bts = [load_bt(0), load_bt(1)]

        # chains for slot k are emitted as filler inside slot k-2
        fillers = {
            0: [lambda: qk_chain(0, 1), lambda: qk_chain(CT, 1),
                lambda: v_chain(1, 0), lambda: v_chain(1, 1),
                lambda: v_chain(1, 2), lambda: qk_chain(1, 0),
                lambda: qk_chain(CT + 1, 0)],
            1: [lambda: v_chain(1, 3), lambda: v_chain(1, 4),
                lambda: qk_chain(1, 1), lambda: qk_chain(CT + 1, 1)],
        }
        for s in range(2, 10):
            hp_n, b_n = (s + 2) // 2, (s + 2) % 2
            jt = hp_n
            fillers[s] = [
                (lambda j=jt, bb=b_n: qk_chain(j, bb)),
                (lambda j=CT + jt, bb=b_n: qk_chain(j, bb)),
            ]
        fillers[6].append(load_wp)
        fillers[10] = []
        fillers[11] = []

        for s in range(12):
            hp, b = s // 2, s % 2
            if b == 0 and hp + 1 < HP and len(bts) <= hp + 1:
                bts.append(load_bt(hp + 1))
            slot(hp, b, bts[hp], fillers[s])

        # ---------------- output projection ----------------
        ot_flat = ot_sb[:].rearrange("p c b n -> p c (b n)")
        ntt = (TP + 127) // 128
        for tt in range(ntt):
            tp = min(128, TP - tt * 128)
            ps = cps.tile([128, 1024], f32, tag="c")
            for ct in range(CT):
                for (j0, jn) in _chunks(C):
                    nc.tensor.matmul(
                        ps[0:tp, j0:j0 + jn],
                        lhsT=ot_flat[:, ct, tt * 128:tt * 128 + tp],
                        rhs=wp_sb[:, ct, j0:j0 + jn],
                        start=(ct == 0),
                        stop=(ct == CT - 1),
                    )
            os = outp.tile([128, C], bf16)
            nc.vector.tensor_add(os[0:tp], ps[0:tp, 0:C], pb_sb[0:tp])
            ring(tt).dma_start(
                out=out_d[tt * 128:tt * 128 + tp, :], in_=os[0:tp]
            )

    nc.compile()
    return nc


def _get_nc():
    if "nc" not in _CACHE:
        _CACHE["nc"] = _build()
    return _CACHE["nc"]


def _prep_inputs(x, relative_position_bias, qkv_weight, q_bias, v_bias,
                 proj_weight, proj_bias):
    """Host-side layout prep + per-core sharding. Returns list of in_maps."""
    import ml_dtypes

    f = np.float32
    bf = ml_dtypes.bfloat16
    x = np.asarray(x, f)
    bias = np.asarray(relative_position_bias, f)
    w = np.asarray(qkv_weight, f)
    qb = np.asarray(q_bias, f)
    vb = np.asarray(v_bias, f)
    wp = np.asarray(proj_weight, f)
    pb = np.asarray(proj_bias, f)

    wq_s = w[0:C] * f(SCALE)            # fold q scaling into weights/bias
    qb_s = qb * f(SCALE)
    wqkT = np.concatenate([wq_s, w[C:2 * C]], 0).T.astype(bf)   # [C, 2C]
    # partition-major jt-blocked layout: [p, jt, ct, jc]
    wqkH = np.ascontiguousarray(
        wqkT.reshape(CT, 128, 2 * CT, 128).transpose(1, 2, 0, 3)
    )
    wvT = np.ascontiguousarray(w[2 * C:].T.astype(bf))
    wpT = np.ascontiguousarray(wp.T.astype(bf))
    qbT = np.ascontiguousarray(qb_s.reshape(CT, 128).T)
    vbB = np.ascontiguousarray(np.broadcast_to(vb, (128, C)))
    pbB = np.ascontiguousarray(np.broadcast_to(pb, (128, C)))
    BT = np.zeros((H, N, NBP), dtype=bf)
    BT[:, :, 0:N] = np.exp(bias.transpose(0, 2, 1), dtype=np.float32).astype(bf)

    shared = dict(wqkH=wqkH, wvT=wvT, wpT=wpT, qbT=qbT, vbB=vbB, pbB=pbB, BT=BT)
    in_maps = []
    for c in range(NCORES):
        xs = x[c * BL:(c + 1) * BL].reshape(T, C)
        in_maps.append(dict(shared, xT=np.ascontiguousarray(xs.T.astype(bf))))
    return in_maps


def kernel(x, relative_position_bias, qkv_weight, q_bias, v_bias,
           proj_weight, proj_bias):
    from concourse import bass_utils

    in_maps = _prep_inputs(x, relative_position_bias, qkv_weight, q_bias,
                           v_bias, proj_weight, proj_bias)
    nc = _get_nc()
    res = bass_utils.run_bass_kernel_spmd(nc, in_maps, core_ids=list(range(NCORES)))
    out = np.concatenate(
        [res.results[c]["out"].reshape(BL, NP, C)[:, :N, :] for c in range(NCORES)],
        axis=0,
    )
    return out.astype(np.float32)


# revision 13
# speedup vs baseline: 1.0976x; 1.0586x over previous
"""Trainium2 Bass kernel for nn_Attention (ViT-style attention block).

Reference computation (per batch b, head h):
    qkv  = x @ qkv_weight.T + [q_bias, 0, v_bias]        # [B,N,3C]
    q,k,v split into heads of HD=64;  q *= HD**-0.5
    S    = q @ k.T + relative_position_bias[h]           # [N,N]
    P    = softmax(S, axis=-1)
    O    = P @ v                                         # [N,HD]
    out  = concat_heads(O) @ proj_weight.T + proj_bias   # [B,N,C]

Sharding: pure data-parallel over batch: 16 batches -> 2 per core across
8 NeuronCores; weights replicated; outputs concatenated on the host.

Single software-pipelined instruction stream (v2): qkv projection chains,
attention head-pair slots and the output projection are emitted in one
interleaved order so the PE never idles long enough to drop out of its
full-speed p-state (PE runs at 1.2GHz for ~3us after any idle gap; the
baseline's phase-separated version paid ~2x on every attention matmul).

Per attention slot (head-pair hp, batch b), queries split 512 + 65 so
every PSUM access stays inside one 2KB bank:
  - S^T pair matmuls (K=64) for both heads at tile_position (0,0)/(64,0).
  - ONE exp on ScalarE per mt covers both heads; P^T *= exp(bias^T)
    (host-precomputed) split between DVE and Pool.
  - O'^T accumulates over key tiles; v_aug = [1 | v] puts the softmax
    denominator in PSUM partition 0, so DVE reciprocal reads it directly
    (no SWDGE collect); f32 HWDGE bounce + partition-broadcast; normalization is an in-place multiply on Pool.
  - qkv/proj chains for FUTURE slots are emitted inside each slot as PE
    filler while ScalarE (exp, the per-slot rate limiter) catches up.

PSUM budget (8 banks): chain pool 2x[128,1024]f32 (4) + S-pair pool
2x[128,2,512]f32 (4); O accumulators borrow chain-pool tiles.

bf16 DMA rule learned on HW: a DMA write run into SBUF must start 4-byte
aligned and may overhang its end by 2 bytes -- all bf16 destination rows
here are padded so overhangs land in padding (580-wide bias rows, 578-wide
ot rows, 66-wide tail tiles).
"""

import numpy as np

B, N, C = 16, 577, 768
H, HD = 12, 64
SCALE = HD ** -0.5
NCORES = 8
BL = B // NCORES           # batches per core (2)
T = BL * N                 # tokens per core (1154)
NT_M = (N + 127) // 128    # key tiles per batch (5: 4x128 + 65)
CT = C // 128              # 128-contraction tiles over C (6)
HP = H // 2                # head pairs (6)
NP = N + 1                 # padded query stride (578)
TP = BL * NP               # padded token rows (1156)
NQ0 = 512                  # main-pass queries per batch
NQ1 = N - NQ0              # tail-pass queries (65)
NBP = 580                  # bias row stride (577 padded to 4B-aligned even)

_CACHE = {}


def _chunks(total, limit=512):
    out = []
    pos = 0
    while pos < total:
        n = min(limit, total - pos)
        out.append((pos, n))
        pos += n
    return out


def _build():
    """Trace the Bass/Tile program once. Returns the Bass object."""
    import concourse.bass as bass
    import concourse.tile as tile
    from concourse import bacc, mybir
    from contextlib import ExitStack

    f32 = mybir.dt.float32
    bf16 = mybir.dt.bfloat16
    ALU = mybir.AluOpType
    ACTF = mybir.ActivationFunctionType

    nc = bacc.Bacc("TRN2", target_bir_lowering=False, debug=False)

    # ---- DRAM I/O ----
    xT_d = nc.dram_tensor("xT", [C, T], bf16, kind="ExternalInput").ap()
    # qk weights partition-major + jt-block-major: one contiguous DMA per jt
    wqk_d = nc.dram_tensor(
        "wqkH", [128, 2 * CT, CT, 128], bf16, kind="ExternalInput"
    ).ap()
    wv_d = nc.dram_tensor("wvT", [C, C], bf16, kind="ExternalInput").ap()
    wp_d = nc.dram_tensor("wpT", [C, C], bf16, kind="ExternalInput").ap()
    qb_d = nc.dram_tensor("qbT", [128, CT], f32, kind="ExternalInput").ap()
    vb_d = nc.dram_tensor("vbB", [128, C], f32, kind="ExternalInput").ap()
    pb_d = nc.dram_tensor("pbB", [128, C], f32, kind="ExternalInput").ap()
    bt_d = nc.dram_tensor("BT", [H, N, NBP], bf16, kind="ExternalInput").ap()
    out_d = nc.dram_tensor("out", [TP, C], bf16, kind="ExternalOutput").ap()

    with tile.TileContext(nc) as tc, ExitStack() as ctx:
        const = ctx.enter_context(tc.tile_pool(name="const", bufs=1))
        persist = ctx.enter_context(tc.tile_pool(name="persist", bufs=1))

        qb_sb = const.tile([128, CT], f32)
        vb_sb = const.tile([128, C], f32)
        pb_sb = const.tile([128, C], f32)

        xT_sb = persist.tile([128, CT, T], bf16)
        wqk_sb = persist.tile([128, 2 * CT, CT, 128], bf16)
        wv_sb = persist.tile([128, CT, C], bf16)
        wp_sb = persist.tile([128, CT, C], bf16)
        qk_sb = persist.tile([128, 2 * CT, BL, NP], bf16)   # q^T | k^T
        v_sb = persist.tile([128, BL * NT_M, H, HD + 2], bf16)  # [1|v|pad]
        ot_sb = persist.tile([128, HP, BL, NP], bf16)       # O^T normalized

        # SBUF pools
        btp = ctx.enter_context(tc.tile_pool(name="btp", bufs=2))
        ptp = ctx.enter_context(tc.tile_pool(name="ptp", bufs=8))
        pttp = ctx.enter_context(tc.tile_pool(name="pttp", bufs=2))
        scp = ctx.enter_context(tc.tile_pool(name="scp", bufs=2))
        sctp = ctx.enter_context(tc.tile_pool(name="sctp", bufs=2))
        rap = ctx.enter_context(tc.tile_pool(name="rap", bufs=2))
        rdp = ctx.enter_context(tc.tile_pool(name="rdp", bufs=2, space="DRAM"))
        bcp = ctx.enter_context(tc.tile_pool(name="bcp", bufs=3))
        outp = ctx.enter_context(tc.tile_pool(name="outp", bufs=3))

        # PSUM pools: chains+O accumulators (2-bank tiles) and S pairs
        cps = ctx.enter_context(tc.tile_pool(name="cps", bufs=2, space="PSUM"))
        sps = ctx.enter_context(tc.tile_pool(name="sps", bufs=2, space="PSUM"))

        # ---------------- input DMAs (need-ordered) ----------------
        def ring(i):
            return nc.sync if i % 2 == 0 else nc.scalar

        # x^T for batch 0 first, then the first head-pair's qk weights
        for ct in range(CT):
            ring(ct).dma_start(
                out=xT_sb[:, ct, 0:N], in_=xT_d[ct * 128:(ct + 1) * 128, 0:N]
            )

        def load_wqk(jt):
            ring(jt).dma_start(out=wqk_sb[:, jt], in_=wqk_d[:, jt])

        load_wqk(0)
        load_wqk(CT)
        for ct in range(CT):
            ring(ct).dma_start(
                out=wv_sb[:, ct, :], in_=wv_d[ct * 128:(ct + 1) * 128, :]
            )
        for ct in range(CT):
            ring(ct).dma_start(
                out=xT_sb[:, ct, N:T], in_=xT_d[ct * 128:(ct + 1) * 128, N:T]
            )
        load_wqk(1)
        load_wqk(CT + 1)
        nc.sync.dma_start(out=qb_sb[:], in_=qb_d)
        nc.scalar.dma_start(out=vb_sb[:], in_=vb_d)
        # ones column of v_aug (denominator trick): partition-0 output row
        nc.gpsimd.memset(v_sb[:, :, :, 0:1], 1.0)
        # ot pad column: keeps pad-token proj rows finite
        nc.gpsimd.memset(ot_sb[:, :, :, N:NP], 0.0)

        def load_bt(hp):
            """exp(bias)^T for head pair hp: [key-part, par, key-tile, q]."""
            bt = btp.tile([128, 2, NT_M, NBP], bf16)
            for par in range(2):
                h = 2 * hp + par
                ring(par).dma_start(
                    out=bt[:, par, 0:4, 0:N],
                    in_=bt_d[h, 0:512, 0:N].rearrange("(a p) n -> p a n", p=128),
                )
                ring(par + 1).dma_start(
                    out=bt[0:65, par, 4, 0:N], in_=bt_d[h, 512:N, 0:N]
                )
            return bt

        # ---------------- chains ----------------
        def qk_chain(jt, b):
            ps = cps.tile([128, 1024], f32, tag="c")
            for ct in range(CT):
                for (q0, qn) in _chunks(N):
                    nc.tensor.matmul(
                        ps[:, q0:q0 + qn],
                        lhsT=wqk_sb[:, jt, ct, :],
                        rhs=xT_sb[:, ct, b * N + q0:b * N + q0 + qn],
                        start=(ct == 0),
                        stop=(ct == CT - 1),
                    )
            if jt < CT:
                nc.vector.tensor_scalar(
                    out=qk_sb[:, jt, b, 0:N],
                    in0=ps[:, 0:N],
                    scalar1=qb_sb[:, jt:jt + 1],
                    scalar2=None,
                    op0=ALU.add,
                )
            else:
                # k copies on ScalarE: DVE is the loaded engine
                nc.scalar.activation(qk_sb[:, jt, b, 0:N], ps[:, 0:N], ACTF.Copy)

        vb_v = vb_sb[:].rearrange("p (h d) -> p h d", d=HD)

        def v_chain(b, mt):
            mp = min(128, N - mt * 128)
            t0 = b * N + mt * 128
            ps = cps.tile([128, 1024], f32, tag="c")
            for ct in range(CT):
                for (j0, jn) in _chunks(C):
                    nc.tensor.matmul(
                        ps[0:mp, j0:j0 + jn],
                        lhsT=xT_sb[:, ct, t0:t0 + mp],
                        rhs=wv_sb[:, ct, j0:j0 + jn],
                        start=(ct == 0),
                        stop=(ct == CT - 1),
                    )
            nc.vector.tensor_add(
                v_sb[0:mp, b * NT_M + mt, :, 1:HD + 1],
                ps[0:mp, 0:C].rearrange("p (h d) -> p h d", d=HD),
                vb_v[0:mp],
            )

        def load_wp():
            nc.scalar.dma_start(
                out=wp_sb[:], in_=wp_d.rearrange("(a p) j -> p a j", p=128)
            )

        # ---------------- attention slot ----------------
        def slot(hp, b, bt, fillers):
            kT = qk_sb[:, CT + hp, b, :]
            qT = qk_sb[:, hp, b, :]
            # S main (queries 0:512): K=64 pairs on PE row groups 0/64
            pts = []
            for mt in range(NT_M):
                mp = min(128, N - mt * 128)
                sp = sps.tile([128, 2, NQ0], f32, tag="sp")
                for par in range(2):
                    p0 = 64 * par
                    nc.tensor.matmul(
                        sp[0:mp, par, :],
                        lhsT=kT[p0:p0 + 64, mt * 128:mt * 128 + mp],
                        rhs=qT[p0:p0 + 64, 0:NQ0],
                        start=True,
                        stop=True,
                        tile_position=(p0, 0),
                    )
                pt = ptp.tile([128, 2, NQ0], bf16)
                nc.scalar.activation(pt[0:mp], sp[0:mp], ACTF.Exp)
                # bias mult: split DVE/Pool to balance engine load
                eng = nc.vector if mt % 2 == 0 else nc.gpsimd
                eng.tensor_mul(pt[0:mp], pt[0:mp], bt[0:mp, :, mt, 0:NQ0])
                pts.append(pt)

            for f in fillers:
                f()

            # O main accumulation; partition 0 = softmax denominator
            ou = cps.tile([128, 1024], f32, name="ou", tag="c")
            ou = ou.rearrange("p (a n) -> p a n", n=NQ0)
            for mt in range(NT_M):
                mp = min(128, N - mt * 128)
                for par in range(2):
                    nc.tensor.matmul(
                        ou[0:HD + 1, par, :],
                        lhsT=v_sb[0:mp, b * NT_M + mt, 2 * hp + par, 0:HD + 1],
                        rhs=pts[mt][0:mp, par, :],
                        start=(mt == 0),
                        stop=(mt == NT_M - 1),
                    )

            # S tail (queries 512:577): all 5 key tiles in one PSUM tile
            spt = sps.tile([128, 2, NQ0], f32, tag="sp")
            for mt in range(NT_M):
                mp = min(128, N - mt * 128)
                for par in range(2):
                    p0 = 64 * par
                    nc.tensor.matmul(
                        spt[0:mp, par, mt * NQ1:(mt + 1) * NQ1],
                        lhsT=kT[p0:p0 + 64, mt * 128:mt * 128 + mp],
                        rhs=qT[p0:p0 + 64, NQ0:N],
                        start=True,
                        stop=True,
                        tile_position=(p0, 0),
                    )
            ptt = pttp.tile([128, 2, NT_M * NQ1], bf16)
            ptt_v = ptt[:].rearrange("p a (m q) -> p a m q", q=NQ1)
            nc.scalar.activation(
                ptt[:, :, 0:4 * NQ1], spt[:, :, 0:4 * NQ1], ACTF.Exp
            )
            nc.scalar.activation(
                ptt[0:65, :, 4 * NQ1:5 * NQ1],
                spt[0:65, :, 4 * NQ1:5 * NQ1],
                ACTF.Exp,
            )
            nc.vector.tensor_mul(
                ptt_v[:, :, 0:4, :], ptt_v[:, :, 0:4, :], bt[:, :, 0:4, NQ0:N]
            )
            nc.vector.tensor_mul(
                ptt_v[0:65, :, 4, :], ptt_v[0:65, :, 4, :],
                bt[0:65, :, 4, NQ0:N],
            )
            out_t = cps.tile([128, 1024], f32, name="out_t", tag="c")
            out_t = out_t.rearrange("p (a n) -> p a n", n=NQ0)
            for mt in range(NT_M):
                mp = min(128, N - mt * 128)
                for par in range(2):
                    nc.tensor.matmul(
                        out_t[0:HD + 1, par, 0:NQ1],
                        lhsT=v_sb[0:mp, b * NT_M + mt, 2 * hp + par, 0:HD + 1],
                        rhs=ptt[0:mp, par, mt * NQ1:(mt + 1) * NQ1],
                        start=(mt == 0),
                        stop=(mt == NT_M - 1),
                    )

            # reciprocals of the denominator rows (PSUM partition 0);
            # main+tail packed contiguously so ONE bounce+broadcast covers
            # all 577 queries (Pool has no divide opcode - verified)
            rall = rap.tile([1, 2, NBP], f32)
            nc.vector.reciprocal_approx_fast(rall[0:1, :, 0:NQ0], ou[0:1, :, :])
            nc.vector.reciprocal_approx_fast(
                rall[0:1, :, NQ0:N], out_t[0:1, :, 0:NQ1]
            )
            # PSUM -> SBUF drains (DVE casts f32->bf16)
            sc = scp.tile([128, 2, NQ0], bf16)
            nc.vector.tensor_copy(sc[0:HD + 1], ou[0:HD + 1])
            sct = sctp.tile([128, 2, HD + 2], bf16)
            nc.vector.tensor_copy(
                sct[0:HD + 1, :, 0:NQ1], out_t[0:HD + 1, :, 0:NQ1]
            )
            rd = rdp.tile([2, NBP], f32)
            nc.sync.dma_start(out=rd[:, 0:N], in_=rall[0:1, :, 0:N])
            bc = bcp.tile([128, NBP], f32)
            for par in range(2):
                p0 = 64 * par
                ring(par).dma_start(
                    out=bc[p0:p0 + 64, 0:N],
                    in_=rd[par:par + 1, 0:N].broadcast_to([64, N]),
                )
                # head-half reorder: sc rows 1:65 -> ot partitions
                ring(par).dma_start(
                    out=ot_sb[p0:p0 + 64, hp, b, 0:NQ0], in_=sc[1:HD + 1, par, :]
                )
                ring(par + 1).dma_start(
                    out=ot_sb[p0:p0 + 64, hp, b, NQ0:N],
                    in_=sct[1:HD + 1, par, 0:NQ1],
                )
            # normalize in place on Pool (SBUF-only engine)
            nc.gpsimd.tensor_mul(
                ot_sb[:, hp, b, 0:N], ot_sb[:, hp, b, 0:N], bc[:, 0:N]
            )

        # ---------------- emission schedule ----------------
        # prologue chains: everything slot (0,0) and (0,1) need
        qk_chain(0, 0)
        qk_chain(CT, 0)
        for mt in range(NT_M):
            v_chain(0, mt)
        bts = [load_bt(0), load_bt(1)]
        for jt in range(2, CT):
            load_wqk(jt)
            load_wqk(CT + jt)
        nc.sync.dma_start(out=pb_sb[:], in_=pb_d)

        # chains for slot k are emitted as filler inside slot k-2
        fillers = {
            0: [lambda: qk_chain(0, 1), lambda: qk_chain(CT, 1),
                lambda: v_chain(1, 0), lambda: v_chain(1, 1),
                lambda: v_chain(1, 2), lambda: qk_chain(1, 0),
                lambda: qk_chain(CT + 1, 0)],
            1: [lambda: v_chain(1, 3), lambda: v_chain(1, 4),
                lambda: qk_chain(1, 1), lambda: qk_chain(CT + 1, 1)],
        }
        for s in range(2, 10):
            hp_n, b_n = (s + 2) // 2, (s + 2) % 2
            jt = hp_n
            fillers[s] = [
                (lambda j=jt, bb=b_n: qk_chain(j, bb)),
                (lambda j=CT + jt, bb=b_n: qk_chain(j, bb)),
            ]
        fillers[6].append(load_wp)
        fillers[10] = []
        fillers[11] = []

        for s in range(12):
            hp, b = s // 2, s % 2
            if b == 0 and hp + 1 < HP and len(bts) <= hp + 1:
                bts.append(load_bt(hp + 1))
            slot(hp, b, bts[hp], fillers[s])

        # ---------------- output projection ----------------
        ot_flat = ot_sb[:].rearrange("p c b n -> p c (b n)")
        ntt = (TP + 127) // 128
        for tt in range(ntt):
            tp = min(128, TP - tt * 128)
            ps = cps.tile([128, 1024], f32, tag="c")
            for ct in range(CT):
                for (j0, jn) in _chunks(C):
                    nc.tensor.matmul(
                        ps[0:tp, j0:j0 + jn],
                        lhsT=ot_flat[:, ct, tt * 128:tt * 128 + tp],
                        rhs=wp_sb[:, ct, j0:j0 + jn],
                        start=(ct == 0),
                        stop=(ct == CT - 1),
                    )
            os = outp.tile([128, C], bf16)
            nc.vector.tensor_add(os[0:tp], ps[0:tp, 0:C], pb_sb[0:tp])
            ring(tt).dma_start(
                out=out_d[tt * 128:tt * 128 + tp, :], in_=os[0:tp]
            )

    nc.compile()
    return nc


def _get_nc():
    if "nc" not in _CACHE:
        _CACHE["nc"] = _build()
    return _CACHE["nc"]


def _prep_inputs(x, relative_position_bias, qkv_weight, q_bias, v_bias,
                 proj_weight, proj_bias):
    """Host-side layout prep + per-core sharding. Returns list of in_maps."""
    import ml_dtypes

    f = np.float32
    bf = ml_dtypes.bfloat16
    x = np.asarray(x, f)
    bias = np.asarray(relative_position_bias, f)
    w = np.asarray(qkv_weight, f)
    qb = np.asarray(q_bias, f)
    vb = np.asarray(v_bias, f)
    wp = np.asarray(proj_weight, f)
    pb = np.asarray(proj_bias, f)

    wq_s = w[0:C] * f(SCALE)            # fold q scaling into weights/bias
    qb_s = qb * f(SCALE)
    wqkT = np.concatenate([wq_s, w[C:2 * C]], 0).T.astype(bf)   # [C, 2C]
    # partition-major jt-blocked layout: [p, jt, ct, jc]
    wqkH = np.ascontiguousarray(
        wqkT.reshape(CT, 128, 2 * CT, 128).transpose(1, 2, 0, 3)
    )
    wvT = np.ascontiguousarray(w[2 * C:].T.astype(bf))
    wpT = np.ascontiguousarray(wp.T.astype(bf))
    qbT = np.ascontiguousarray(qb_s.reshape(CT, 128).T)
    vbB = np.ascontiguousarray(np.broadcast_to(vb, (128, C)))
    pbB = np.ascontiguousarray(np.broadcast_to(pb, (128, C)))
    BT = np.zeros((H, N, NBP), dtype=bf)
    BT[:, :, 0:N] = np.exp(bias.transpose(0, 2, 1), dtype=np.float32).astype(bf)

    shared = dict(wqkH=wqkH, wvT=wvT, wpT=wpT, qbT=qbT, vbB=vbB, pbB=pbB, BT=BT)
    in_maps = []
    for c in range(NCORES):
        xs = x[c * BL:(c + 1) * BL].reshape(T, C)
        in_maps.append(dict(shared, xT=np.ascontiguousarray(xs.T.astype(bf))))
    return in_maps


def kernel(x, relative_position_bias, qkv_weight, q_bias, v_bias,
           proj_weight, proj_bias):
    from concourse import bass_utils

    in_maps = _prep_inputs(x, relative_position_bias, qkv_weight, q_bias,
                           v_bias, proj_weight, proj_bias)
    nc = _get_nc()
    res = bass_utils.run_bass_kernel_spmd(nc, in_maps, core_ids=list(range(NCORES)))
    out = np.concatenate(
        [res.results[c]["out"].reshape(BL, NP, C)[:, :N, :] for c in range(NCORES)],
        axis=0,
    )
    return out.astype(np.float32)
